# revision 3
# baseline (speedup 1.0000x reference)
"""Trainium2 Bass kernel for nn_DifferentialNetwork (ensemble MLP with
input-Jacobian and input-Hessian outputs).

Reference computes, per network n (4 nets, batch 512, width 256, n_in 12):
  3 tanh layers + linear head, propagating value h, Jacobian dh [B,256,12]
  and full Hessian d2h [B,12,256,12] through every layer (~80 GFLOP).

Because the head output is SCALAR per network, the Hessian has the closed
form   H_b = sum_l J_l^T diag(u_l * g''(a_l)) J_l   (one term per tanh
layer), where J_l = da_l/dx and u_l = dout/dt_l.  Evaluated backward with
fixed-weight matmuls only:

  R2 = c2*J2 ; S2 = W2^T R2 ; U1 = c1*J1 + gp1*S2 ; S1 = W1^T U1
  U0 = c0*W0 + gp0*S1 ;  H = -2 * W0^T U0
  (c_l = u_l * t_l * gp_l, the -2 of g'' = -2*t*g' factored into the end)

This needs ~4 GFLOP total instead of ~80, and every matmul has a
per-network FIXED stationary operand with batch*12 free columns - ideal
for the PE array.

Sharding: 8 cores = 4 networks x 2 batch halves (B_local = 256 per core).
Weights per core are that network's only; host preps transposed /
partition-split layouts so no on-device transposes are needed.

Matmuls run in float32r (full-rate PE streaming for N>=256, vs 4x slower
plain fp32); every matmul operand tensor is declared float32r so producers
round (walrus birverifier requirement).
"""

from contextlib import ExitStack, nullcontext

import numpy as np

import concourse.bacc as bacc
import concourse.mybir as mybir
import concourse.tile as tile
from concourse.bass_utils import run_bass_kernel_spmd

N_NET, B_FULL, N_IN, W = 4, 512, 12, 256
N_CORES = 8
BL = 256          # batch per core
CB = 32           # chunk batch for the Hessian chain
NCH = BL // CB
JC = CB * N_IN    # 384 free columns per chunk

F32 = mybir.dt.float32
F32R = mybir.dt.float32r
AF = mybir.ActivationFunctionType
OP = mybir.AluOpType

# engine assignment for the per-chunk elementwise ops (rebalance knob)
ENG = {"dh0": "gpsimd", "c0w0": "gpsimd", "r2": "gpsimd",
       "dh1": "vector", "c1j1": "vector", "dh2": "vector",
       "gs2": "vector", "gs1": "vector"}


def build(mm_dt=F32R, iters=1, eng=None):
    """Build + compile the per-core Bass program (identical on all cores)."""
    eng = dict(ENG, **(eng or {}))
    nc = bacc.Bacc("TRN2", num_devices=N_CORES)

    def E(key):
        return getattr(nc, eng[key])

    # matmul-operand inputs: declared mm_dt so the dtype chain is consistent
    d_xT = nc.dram_tensor("xT", [N_IN, BL], mm_dt, kind="ExternalInput").ap()
    d_w0T = nc.dram_tensor("w0T", [N_IN, W], mm_dt, kind="ExternalInput").ap()
    d_w0 = nc.dram_tensor("w0", [128, 2, N_IN], mm_dt, kind="ExternalInput").ap()
    d_w1T = nc.dram_tensor("w1T", [128, 2, W], mm_dt, kind="ExternalInput").ap()
    d_w2T = nc.dram_tensor("w2T", [128, 2, W], mm_dt, kind="ExternalInput").ap()
    d_w1 = nc.dram_tensor("w1", [128, 2, W], mm_dt, kind="ExternalInput").ap()
    d_w2 = nc.dram_tensor("w2", [128, 2, W], mm_dt, kind="ExternalInput").ap()
    d_w3T = nc.dram_tensor("w3T", [128, 2, 1], mm_dt, kind="ExternalInput").ap()
    # f32 copies for elementwise broadcast reads
    d_w0c = nc.dram_tensor("w0c", [128, 2, N_IN], F32, kind="ExternalInput").ap()
    d_w3c = nc.dram_tensor("w3c", [128, 2, 1], F32, kind="ExternalInput").ap()
    d_b0 = nc.dram_tensor("b0", [128, 2, 1], F32, kind="ExternalInput").ap()
    d_b1 = nc.dram_tensor("b1", [128, 2, 1], F32, kind="ExternalInput").ap()
    d_b2 = nc.dram_tensor("b2", [128, 2, 1], F32, kind="ExternalInput").ap()
    d_b3 = nc.dram_tensor("b3", [1, 1], F32, kind="ExternalInput").ap()

    d_val = nc.dram_tensor("o_val", [BL, 1], F32, kind="ExternalOutput").ap()
    d_jac = nc.dram_tensor("o_jac", [BL, N_IN], F32, kind="ExternalOutput").ap()
    d_hes = nc.dram_tensor("o_hes", [BL, N_IN, N_IN], F32, kind="ExternalOutput").ap()

    with tile.TileContext(nc) as tc, ExitStack() as ctx:
        wp = ctx.enter_context(tc.tile_pool(name="wp", bufs=1))
        fw = ctx.enter_context(tc.tile_pool(name="fw", bufs=1))
        cs = ctx.enter_context(tc.tile_pool(name="cs", bufs=3))
        outp = ctx.enter_context(tc.tile_pool(name="outp", bufs=2))

        loop = tc.For_i(0, iters, 1, name="rep") if iters > 1 else nullcontext()
        with loop:
            # ---- load inputs ----
            def load(name, shape, dt, dram):
                t = wp.tile(shape, dt, tag=name, name=name)
                nc.sync.dma_start(out=t[:], in_=dram[:])
                return t

            xT = load("xTs", [N_IN, BL], mm_dt, d_xT)
            w0T = load("w0Ts", [N_IN, W], mm_dt, d_w0T)
            w0 = load("w0s", [128, 2, N_IN], mm_dt, d_w0)
            w1T = load("w1Ts", [128, 2, W], mm_dt, d_w1T)
            w2T = load("w2Ts", [128, 2, W], mm_dt, d_w2T)
            w1 = load("w1s", [128, 2, W], mm_dt, d_w1)
            w2 = load("w2s", [128, 2, W], mm_dt, d_w2)
            w3T = load("w3Ts", [128, 2, 1], mm_dt, d_w3T)
            w0c = load("w0cs", [128, 2, N_IN], F32, d_w0c)
            w3c = load("w3cs", [128, 2, 1], F32, d_w3c)
            b0 = load("b0s", [128, 2, 1], F32, d_b0)
            b1 = load("b1s", [128, 2, 1], F32, d_b1)
            b2 = load("b2s", [128, 2, 1], F32, d_b2)
            b3 = load("b3s", [1, 1], F32, d_b3)

            # ---- forward + backward-gradient phase (full B) ----
            t_l = [
                fw.tile([128, 2, BL], mm_dt, tag=f"t{l}", name=f"t{l}")
                for l in range(3)
            ]
            gp_l = [
                fw.tile([128, 2, BL], F32, tag=f"gp{l}", name=f"gp{l}")
                for l in range(3)
            ]
            tgp0 = fw.tile([128, 2, BL], F32, tag="tgp0")
            tgp1 = fw.tile([128, 2, BL], F32, tag="tgp1")
            gp2u2 = fw.tile([128, 2, BL], mm_dt, tag="gp2u2")
            t2u2 = fw.tile([128, 2, BL], F32, tag="t2u2")
            c1h = fw.tile([128, 2, BL], F32, tag="c1h")
            gp1u1 = fw.tile([128, 2, BL], mm_dt, tag="gp1u1")
            c0h = fw.tile([128, 2, BL], F32, tag="c0h")

            with tc.tile_pool(name="fps", bufs=2, space="PSUM") as fps:
                # layer 0: K = N_IN
                for m in range(2):
                    ap_ = fps.tile([128, BL], F32, tag="a", name="a")
                    nc.tensor.matmul(
                        ap_[:], w0T[:, m * 128:(m + 1) * 128], xT[:],
                        start=True, stop=True,
                    )
                    nc.scalar.activation(
                        t_l[0][:, m, :], ap_[:], AF.Tanh, bias=b0[:, m, :]
                    )
                # layers 1, 2
                for l, (wT, bs) in enumerate(((w1T, b1), (w2T, b2)), start=1):
                    for m in range(2):
                        ap_ = fps.tile([128, BL], F32, tag="a", name="a")
                        for p in range(2):
                            nc.tensor.matmul(
                                ap_[:],
                                wT[:, p, m * 128:(m + 1) * 128],
                                t_l[l - 1][:, p, :],
                                start=(p == 0), stop=(p == 1),
                            )
                        nc.scalar.activation(
                            t_l[l][:, m, :], ap_[:], AF.Tanh, bias=bs[:, m, :]
                        )
                # value head: w3 . t2 + b3
                op_ = fps.tile([1, BL], F32, tag="oh", name="oh")
                for p in range(2):
                    nc.tensor.matmul(
                        op_[:], w3T[:, p, :], t_l[2][:, p, :],
                        start=(p == 0), stop=(p == 1),
                    )
                oval = fw.tile([1, BL], F32, tag="oval")
                nc.vector.tensor_scalar(oval[:], op_[:], b3[:], None, OP.add)
                nc.sync.dma_start(out=d_val.transpose([1, 0]), in_=oval[:])

                # g' = 1 - t^2 and t*g'
                for l in range(3):
                    tsq = fw.tile([128, 2, BL], F32, tag="tsq", name="tsq")
                    nc.scalar.square(tsq[:], t_l[l][:])
                    nc.vector.tensor_scalar(
                        gp_l[l][:], tsq[:], -1.0, 1.0, OP.mult, OP.add
                    )
                nc.vector.tensor_mul(tgp0[:], t_l[0][:], gp_l[0][:])
                nc.vector.tensor_mul(tgp1[:], t_l[1][:], gp_l[1][:])

                # u-chain (u2 = w3 broadcast over batch)
                w3b = w3c[:].broadcast_to([128, 2, BL])
                nc.vector.tensor_tensor(gp2u2[:], gp_l[2][:], w3b, OP.mult)
                nc.vector.tensor_tensor(t2u2[:], t_l[2][:], w3b, OP.mult)
                for m in range(2):
                    up_ = fps.tile([128, BL], F32, tag="a", name="a")
                    for p in range(2):
                        nc.tensor.matmul(
                            up_[:],
                            w2[:, p, m * 128:(m + 1) * 128],
                            gp2u2[:, p, :],
                            start=(p == 0), stop=(p == 1),
                        )
                    nc.vector.tensor_tensor(
                        c1h[:, m, :], up_[:], tgp1[:, m, :], OP.mult
                    )
                    nc.vector.tensor_tensor(
                        gp1u1[:, m, :], up_[:], gp_l[1][:, m, :], OP.mult
                    )
                for m in range(2):
                    up_ = fps.tile([128, BL], F32, tag="a", name="a")
                    for p in range(2):
                        nc.tensor.matmul(
                            up_[:],
                            w1[:, p, m * 128:(m + 1) * 128],
                            gp1u1[:, p, :],
                            start=(p == 0), stop=(p == 1),
                        )
                    nc.vector.tensor_tensor(
                        c0h[:, m, :], up_[:], tgp0[:, m, :], OP.mult
                    )

            # ---- Hessian / Jacobian chain, chunked over batch ----
            with (
                tc.tile_pool(name="cps", bufs=3, space="PSUM") as cps,
                tc.tile_pool(name="hps", bufs=2, space="PSUM") as hps,
            ):
                for c in range(NCH):
                    sl = slice(c * CB, (c + 1) * CB)

                    def bc(v):  # [128,2,CB] -> [128,2,CB,N_IN] broadcast
                        return v[:, :, sl].unsqueeze(3).broadcast_to(
                            [128, 2, CB, N_IN]
                        )

                    w0b = w0c[:].unsqueeze(2).broadcast_to([128, 2, CB, N_IN])

                    dh0 = cs.tile([128, 2, CB, N_IN], mm_dt, tag="dh0")
                    E("dh0").tensor_tensor(dh0[:], bc(gp_l[0]), w0b, OP.mult)
                    c0w0 = cs.tile([128, 2, CB, N_IN], mm_dt, tag="c0w0")
                    E("c0w0").tensor_tensor(c0w0[:], bc(c0h), w0b, OP.mult)

                    # J1 = W1 @ dh0
                    j1p = cps.tile([128, 2, 512], F32, tag="chain", name="j1p")
                    for m in range(2):
                        for p in range(2):
                            nc.tensor.matmul(
                                j1p[:, m, 0:JC],
                                w1T[:, p, m * 128:(m + 1) * 128],
                                dh0[:, p].rearrange("r b j -> r (b j)"),
                                start=(p == 0), stop=(p == 1),
                            )
                    j1v = j1p[:, :, 0:JC].rearrange("r m (b j) -> r m b j", j=N_IN)
                    dh1 = cs.tile([128, 2, CB, N_IN], mm_dt, tag="dh1")
                    E("dh1").tensor_tensor(dh1[:], j1v, bc(gp_l[1]), OP.mult)
                    c1j1 = cs.tile([128, 2, CB, N_IN], mm_dt, tag="c1j1")
                    E("c1j1").tensor_tensor(c1j1[:], j1v, bc(c1h), OP.mult)

                    # J2 = W2 @ dh1
                    j2p = cps.tile([128, 2, 512], F32, tag="chain", name="j2p")
                    for m in range(2):
                        for p in range(2):
                            nc.tensor.matmul(
                                j2p[:, m, 0:JC],
                                w2T[:, p, m * 128:(m + 1) * 128],
                                dh1[:, p].rearrange("r b j -> r (b j)"),
                                start=(p == 0), stop=(p == 1),
                            )
                    j2v = j2p[:, :, 0:JC].rearrange("r m (b j) -> r m b j", j=N_IN)
                    dh2 = cs.tile([128, 2, CB, N_IN], mm_dt, tag="dh2")
                    E("dh2").tensor_tensor(dh2[:], j2v, bc(gp_l[2]), OP.mult)
                    r2 = cs.tile([128, 2, CB, N_IN], mm_dt, tag="r2")
                    E("r2").tensor_tensor(r2[:], dh2[:], bc(t2u2), OP.mult)

                    # jacobian head: w3 . dh2
                    jp = hps.tile([N_IN, 512], F32, tag="h", name="jp")
                    for p in range(2):
                        nc.tensor.matmul(
                            jp[0:1, 0:JC],
                            w3T[:, p, :],
                            dh2[:, p].rearrange("r b j -> r (b j)"),
                            start=(p == 0), stop=(p == 1),
                        )
                    jacs = outp.tile([1, CB, N_IN], F32, tag="jacs")
                    nc.scalar.activation(
                        jacs[:].rearrange("r b j -> r (b j)"), jp[0:1, 0:JC],
                        AF.Copy,
                    )
                    nc.sync.dma_start(out=d_jac[sl, :].unsqueeze(0), in_=jacs[:])

                    # S2 = W2^T @ R2
                    s2p = cps.tile([128, 2, 512], F32, tag="chain", name="s2p")
                    for m in range(2):
                        for p in range(2):
                            nc.tensor.matmul(
                                s2p[:, m, 0:JC],
                                w2[:, p, m * 128:(m + 1) * 128],
                                r2[:, p].rearrange("r b j -> r (b j)"),
                                start=(p == 0), stop=(p == 1),
                            )
                    gs2 = cs.tile([128, 2, CB, N_IN], mm_dt, tag="gs2")
                    E("gs2").tensor_tensor(
                        gs2[:],
                        s2p[:, :, 0:JC].rearrange("r m (b j) -> r m b j", j=N_IN),
                        bc(gp_l[1]), OP.mult,
                    )

                    # S1 = W1^T @ (c1*J1 + gp1*S2)  via PSUM accumulation
                    s1p = cps.tile([128, 2, 512], F32, tag="chain", name="s1p")
                    for m in range(2):
                        for p in range(2):
                            nc.tensor.matmul(
                                s1p[:, m, 0:JC],
                                w1[:, p, m * 128:(m + 1) * 128],
                                c1j1[:, p].rearrange("r b j -> r (b j)"),
                                start=(p == 0), stop=False,
                            )
                        for p in range(2):
                            nc.tensor.matmul(
                                s1p[:, m, 0:JC],
                                w1[:, p, m * 128:(m + 1) * 128],
                                gs2[:, p].rearrange("r b j -> r (b j)"),
                                start=False, stop=(p == 1),
                            )
                    gs1 = cs.tile([128, 2, CB, N_IN], mm_dt, tag="gs1")
                    E("gs1").tensor_tensor(
                        gs1[:],
                        s1p[:, :, 0:JC].rearrange("r m (b j) -> r m b j", j=N_IN),
                        bc(gp_l[0]), OP.mult,
                    )

                    # H = -2 * W0^T @ (gp0*S1 + c0*W0)
                    hp = hps.tile([N_IN, 512], F32, tag="h", name="hp")
                    for p in range(2):
                        nc.tensor.matmul(
                            hp[:, 0:JC], w0[:, p, :],
                            gs1[:, p].rearrange("r b j -> r (b j)"),
                            start=(p == 0), stop=False,
                        )
                    for p in range(2):
                        nc.tensor.matmul(
                            hp[:, 0:JC], w0[:, p, :],
                            c0w0[:, p].rearrange("r b j -> r (b j)"),
                            start=False, stop=(p == 1),
                        )
                    hs = outp.tile([N_IN, CB, N_IN], F32, tag="hs")
                    nc.scalar.activation(
                        hs[:].rearrange("r b j -> r (b j)"), hp[:, 0:JC],
                        AF.Copy, scale=-2.0,
                    )
                    nc.sync.dma_start(
                        out=d_hes[sl].transpose([1, 0, 2]), in_=hs[:]
                    )

    nc.compile()
    return nc


_CACHE = {}


def _prep_core_inputs(x, w0, b0, w1, b1, w2, b2, w3, b3):
    """Per-core input maps: core = net * 2 + half."""
    def split(a):  # [256, F] -> [128, 2, F]
        return np.ascontiguousarray(
            a.reshape(2, 128, *a.shape[1:]).transpose(1, 0, *range(2, a.ndim + 1))
        )

    xT = np.ascontiguousarray(x.T)  # [12, 512]
    maps = []
    for n in range(N_NET):
        per_net = {
            "w0T": np.ascontiguousarray(w0[n].T),
            "w0": split(w0[n]),
            "w1T": split(np.ascontiguousarray(w1[n].T)),
            "w2T": split(np.ascontiguousarray(w2[n].T)),
            "w1": split(w1[n]),
            "w2": split(w2[n]),
            "w3T": split(np.ascontiguousarray(w3[n].T)),
            "b0": split(b0[n][:, None]),
            "b1": split(b1[n][:, None]),
            "b2": split(b2[n][:, None]),
            "b3": b3[n][:, None],
        }
        per_net["w0c"] = per_net["w0"]
        per_net["w3c"] = per_net["w3T"]
        for h in range(2):
            m = dict(per_net)
            m["xT"] = np.ascontiguousarray(xT[:, h * BL:(h + 1) * BL])
            maps.append(m)
    return maps


def run(nc, inputs):
    in_maps = _prep_core_inputs(**inputs)
    res = run_bass_kernel_spmd(nc, in_maps, list(range(N_CORES)))
    out = np.zeros((N_NET, B_FULL, 1, 1), np.float32)
    jac = np.zeros((N_NET, B_FULL, 1, N_IN), np.float32)
    hes = np.zeros((N_NET, B_FULL, N_IN, 1, N_IN), np.float32)
    for n in range(N_NET):
        for h in range(2):
            r = res.results[n * 2 + h]
            sl = slice(h * BL, (h + 1) * BL)
            out[n, sl, 0, 0] = r["o_val"][:, 0]
            jac[n, sl, 0, :] = r["o_jac"]
            hes[n, sl, :, 0, :] = r["o_hes"]
    return out, jac, hes


def kernel(x, w0, b0, w1, b1, w2, b2, w3, b3):
    inputs = {
        "x": np.asarray(x, np.float32),
        "w0": np.asarray(w0, np.float32), "b0": np.asarray(b0, np.float32),
        "w1": np.asarray(w1, np.float32), "b1": np.asarray(b1, np.float32),
        "w2": np.asarray(w2, np.float32), "b2": np.asarray(b2, np.float32),
        "w3": np.asarray(w3, np.float32), "b3": np.asarray(b3, np.float32),
    }
    if "nc" not in _CACHE:
        _CACHE["nc"] = build()
    return run(_CACHE["nc"], inputs)


# revision 13
# speedup vs baseline: 2.3327x; 2.3327x over previous
"""Trainium2 Bass kernel for nn_DifferentialNetwork (ensemble MLP with
input-Jacobian and input-Hessian outputs).

Reference computes, per network n (4 nets, batch 512, width 256, n_in 12):
  3 tanh layers + linear head, propagating value h, Jacobian dh [B,256,12]
  and full Hessian d2h [B,12,256,12] through every layer (~80 GFLOP).

Because the head output is SCALAR per network, the Hessian has the closed
form   H_b = sum_l J_l^T diag(u_l * g''(a_l)) J_l   (one term per tanh
layer), where J_l = da_l/dx and u_l = dout/dt_l.  Evaluated backward with
fixed-weight matmuls only:

  R2 = c2*J2 ; S2 = W2^T R2 ; S1 = W1^T (c1*J1 + gp1*S2)
  H  = -2 * ( W0^T (gp0*S1) + T0 ),   T0[(j,k),b] = W00^T @ c0
  where W00[i,(j,k)] = W0[i,j]*W0[i,k] is host-precomputed, c_l = u_l*t_l*gp_l
  (the -2 of g'' = -2*t*g' factored out), and the T0 term is added on the
  host during unsharding (it comes out in [(j,k), b] layout).

This needs ~4 GFLOP total instead of ~80, and every matmul has a
per-network FIXED stationary operand with batch*12 free columns - ideal
for the PE array.

Sharding: 8 cores = 4 networks x 2 batch halves (B_local = 256 per core).
Weights per core are that network's only; host preps transposed /
partition-split layouts so no on-device transposes are needed.

Matmuls run in float32r (full-rate PE streaming for N>=256, vs 4x slower
plain fp32); every matmul operand tensor is declared float32r so producers
round (walrus birverifier requirement).

The Hessian chain is emitted PHASE-MAJOR (all chunks' J1 matmuls, then all
dh1 scales, ...) so each in-order engine queue matches the pipeline order
and chunks overlap instead of serializing.
"""

from contextlib import ExitStack, nullcontext

import numpy as np

import concourse.bacc as bacc
import concourse.mybir as mybir
import concourse.tile as tile
from concourse.bass_utils import run_bass_kernel_spmd

N_NET, B_FULL, N_IN, W = 4, 512, 12, 256
N_CORES = 8
BL = 256          # batch per core
CB = 32           # chunk batch for the Hessian chain
NCH = BL // CB
JC = CB * N_IN    # 384 free columns per chunk
NJK = N_IN * N_IN

F32 = mybir.dt.float32
F32R = mybir.dt.float32r
AF = mybir.ActivationFunctionType
OP = mybir.AluOpType

# engine assignment for the chunked elementwise ops (rebalance knob)
ENG = {"dh0": "gpsimd", "r2": "vector", "c1j1": "gpsimd",
       "dh1": "vector", "gs2": "vector", "gs1": "vector"}


def build(mm_dt=F32R, iters=1, eng=None):
    """Build + compile the per-core Bass program (identical on all cores)."""
    eng = dict(ENG, **(eng or {}))
    nc = bacc.Bacc("TRN2", num_devices=N_CORES)

    def E(key):
        return getattr(nc, eng[key])

    # matmul-operand inputs: declared mm_dt so the dtype chain is consistent
    d_xT = nc.dram_tensor("xT", [N_IN, BL], mm_dt, kind="ExternalInput").ap()
    d_w0T = nc.dram_tensor("w0T", [N_IN, W], mm_dt, kind="ExternalInput").ap()
    d_w0 = nc.dram_tensor("w0", [128, 2, N_IN], mm_dt, kind="ExternalInput").ap()
    d_w1T = nc.dram_tensor("w1T", [128, 2, W], mm_dt, kind="ExternalInput").ap()
    d_w2T = nc.dram_tensor("w2T", [128, 2, W], mm_dt, kind="ExternalInput").ap()
    d_w1 = nc.dram_tensor("w1", [128, 2, W], mm_dt, kind="ExternalInput").ap()
    d_w2 = nc.dram_tensor("w2", [128, 2, W], mm_dt, kind="ExternalInput").ap()
    d_w3T = nc.dram_tensor("w3T", [128, 2, 1], mm_dt, kind="ExternalInput").ap()
    d_w00 = nc.dram_tensor("w00", [128, 2, NJK], mm_dt, kind="ExternalInput").ap()
    d_w0m2 = nc.dram_tensor("w0m2", [128, 2, N_IN], mm_dt, kind="ExternalInput").ap()
    # f32 copies for elementwise broadcast reads
    d_w0c = nc.dram_tensor("w0c", [128, 2, N_IN], F32, kind="ExternalInput").ap()
    d_w3c = nc.dram_tensor("w3c", [128, 2, 1], F32, kind="ExternalInput").ap()
    d_b0 = nc.dram_tensor("b0", [128, 2, 1], F32, kind="ExternalInput").ap()
    d_b1 = nc.dram_tensor("b1", [128, 2, 1], F32, kind="ExternalInput").ap()
    d_b2 = nc.dram_tensor("b2", [128, 2, 1], F32, kind="ExternalInput").ap()
    d_b3 = nc.dram_tensor("b3", [1, 1], F32, kind="ExternalInput").ap()

    d_val = nc.dram_tensor("o_val", [1, BL], F32, kind="ExternalOutput").ap()
    d_jac = nc.dram_tensor("o_jac", [N_IN, BL], F32, kind="ExternalOutput").ap()
    d_hes = nc.dram_tensor("o_hes", [N_IN, BL, N_IN], F32, kind="ExternalOutput").ap()
    d_t0 = nc.dram_tensor("o_t0", [NJK, BL], F32, kind="ExternalOutput").ap()

    with tile.TileContext(nc) as tc, ExitStack() as ctx:
        wp = ctx.enter_context(tc.tile_pool(name="wp", bufs=1))
        fw = ctx.enter_context(tc.tile_pool(name="fw", bufs=1))
        big = ctx.enter_context(tc.tile_pool(name="big", bufs=1))
        outp = ctx.enter_context(tc.tile_pool(name="outp", bufs=3))

        loop = tc.For_i(0, iters, 1, name="rep") if iters > 1 else nullcontext()
        with loop:
            # ---- load inputs on the two HWDGE queues (sync carries the
            # forward-critical-path tensors in need order; scalar carries
            # everything needed later) ----
            def load(q, name, shape, dt, dram):
                t = wp.tile(shape, dt, tag=name, name=name)
                q.dma_start(out=t[:], in_=dram[:])
                return t

            xT = load(nc.sync, "xTs", [N_IN, BL], mm_dt, d_xT)
            w0T = load(nc.sync, "w0Ts", [N_IN, W], mm_dt, d_w0T)
            b0 = load(nc.sync, "b0s", [128, 2, 1], F32, d_b0)
            w0c = load(nc.sync, "w0cs", [128, 2, N_IN], F32, d_w0c)
            w1T = load(nc.sync, "w1Ts", [128, 2, W], mm_dt, d_w1T)
            b1 = load(nc.sync, "b1s", [128, 2, 1], F32, d_b1)
            w2T = load(nc.sync, "w2Ts", [128, 2, W], mm_dt, d_w2T)
            b2 = load(nc.sync, "b2s", [128, 2, 1], F32, d_b2)
            w3T = load(nc.sync, "w3Ts", [128, 2, 1], mm_dt, d_w3T)
            w3c = load(nc.sync, "w3cs", [128, 2, 1], F32, d_w3c)
            b3 = load(nc.sync, "b3s", [1, 1], F32, d_b3)
            w2 = load(nc.sync, "w2s", [128, 2, W], mm_dt, d_w2)
            w1 = load(nc.sync, "w1s", [128, 2, W], mm_dt, d_w1)
            w0 = load(nc.sync, "w0s", [128, 2, N_IN], mm_dt, d_w0)
            w00 = load(nc.sync, "w00s", [128, 2, NJK], mm_dt, d_w00)
            w0m2 = load(nc.sync, "w0m2s", [128, 2, N_IN], mm_dt, d_w0m2)

            # full-B intermediates of the Hessian chain
            def bigt(name):
                return big.tile([128, 2, BL, N_IN], mm_dt, tag=name, name=name)

            # tag-shared pairs with disjoint lifetimes:
            # dh0 dies at the J1 wave, gs2 is born at the S2 wave;
            # dh1 dies at the J2/c1j1 waves, gs1 is born at the S1 wave.
            dh0 = big.tile([128, 2, BL, N_IN], mm_dt, tag="bigA", name="dh0")
            dh1 = big.tile([128, 2, BL, N_IN], mm_dt, tag="bigB", name="dh1")
            c1j1 = bigt("c1j1")
            r2 = bigt("r2")

            def fwt(name, dt=F32):
                return fw.tile([128, 2, BL], dt, tag=name, name=name)

            t_l = [fwt(f"t{l}", mm_dt) for l in range(3)]
            gp_l = [fwt(f"gp{l}") for l in range(3)]
            tgp0 = fwt("tgp0")
            u1t1 = fwt("u1t1")
            gp1u1 = fwt("gp1u1", mm_dt)
            gp2u2 = fwt("gp2u2", mm_dt)
            c2h = fwt("c2h")
            gp0u0 = fwt("gp0u0", mm_dt)
            c0 = fwt("c0", mm_dt)

            def chunk4(v, c):  # [128,2,BL,N_IN] tensor -> chunk c flat cols
                return v[:, :, c * CB:(c + 1) * CB, :]

            def bcast(v, c):  # [128,2,BL] -> [128,2,CB,N_IN] broadcast
                return v[:, :, c * CB:(c + 1) * CB].unsqueeze(3).broadcast_to(
                    [128, 2, CB, N_IN]
                )

            w0b = None  # set after w0c load

            with (
                tc.tile_pool(name="fps", bufs=2, space="PSUM") as fps,
                tc.tile_pool(name="cps", bufs=2, space="PSUM") as cps,
                tc.tile_pool(name="hps", bufs=2, space="PSUM") as hps,
            ):
                # ---- forward layers ----
                for m in range(2):
                    ap_ = fps.tile([128, BL], F32, tag="a", name="a")
                    nc.tensor.matmul(
                        ap_[:], w0T[:, m * 128:(m + 1) * 128], xT[:],
                        start=True, stop=True,
                    )
                    nc.scalar.activation(
                        t_l[0][:, m, :], ap_[:], AF.Tanh, bias=b0[:, m, :]
                    )
                # gp0 immediately, then launch the dh0 wave on GPSIMD so it
                # overlaps the rest of the forward pass
                tsq = fwt("tsq")
                nc.scalar.square(tsq[:], t_l[0][:])
                nc.vector.tensor_scalar(
                    gp_l[0][:], tsq[:], -1.0, 1.0, OP.mult, OP.add
                )
                w0b = w0c[:].unsqueeze(2).broadcast_to([128, 2, CB, N_IN])
                for c in range(NCH):
                    e = nc.vector if c % 2 == 0 else nc.gpsimd
                    e.tensor_tensor(
                        chunk4(dh0, c), bcast(gp_l[0], c), w0b, OP.mult
                    )
                nc.vector.tensor_mul(tgp0[:], t_l[0][:], gp_l[0][:])

                for l, (wT, bs) in enumerate(((w1T, b1), (w2T, b2)), start=1):
                    for m in range(2):
                        ap_ = fps.tile([128, BL], F32, tag="a", name="a")
                        for p in range(2):
                            nc.tensor.matmul(
                                ap_[:],
                                wT[:, p, m * 128:(m + 1) * 128],
                                t_l[l - 1][:, p, :],
                                start=(p == 0), stop=(p == 1),
                            )
                        nc.scalar.activation(
                            t_l[l][:, m, :], ap_[:], AF.Tanh, bias=bs[:, m, :]
                        )
                    tsq2 = fwt("tsq")
                    nc.scalar.square(tsq2[:], t_l[l][:])
                    nc.vector.tensor_scalar(
                        gp_l[l][:], tsq2[:], -1.0, 1.0, OP.mult, OP.add
                    )

                # value head: w3 . t2 + b3
                op_ = fps.tile([1, BL], F32, tag="a", name="oh")
                for p in range(2):
                    nc.tensor.matmul(
                        op_[:], w3T[:, p, :], t_l[2][:, p, :],
                        start=(p == 0), stop=(p == 1),
                    )
                oval = fw.tile([1, BL], F32, tag="oval")
                nc.vector.tensor_scalar(oval[:], op_[:], b3[:], None, OP.add)
                nc.sync.dma_start(out=d_val[:], in_=oval[:])

                # u2-level links BEFORE the dh1 wave so the u1 matmuls are
                # not blocked behind the DVE drain queue
                w3b = w3c[:].broadcast_to([128, 2, BL])
                nc.vector.tensor_tensor(gp2u2[:], gp_l[2][:], w3b, OP.mult)
                nc.vector.tensor_tensor(c2h[:], gp2u2[:], t_l[2][:], OP.mult)

                # ---- wave machinery ----
                def mm_wave(name, lhsT, rhs_t, mtiles=2):
                    rhs_list = rhs_t if isinstance(rhs_t, list) else [rhs_t]
                    for c in range(NCH):
                        pt = cps.tile(
                            [128, 2, 512], F32, tag="chain", name=f"{name}p"
                        )
                        for m in range(mtiles):
                            first = True
                            for ri, rt in enumerate(rhs_list):
                                last_r = ri == len(rhs_list) - 1
                                for p in range(2):
                                    nc.tensor.matmul(
                                        pt[:, m, 0:JC],
                                        lhsT[:, p, m * 128:(m + 1) * 128],
                                        chunk4(rt, c)[:, p].rearrange(
                                            "r b j -> r (b j)"
                                        ),
                                        start=first, stop=(last_r and p == 1),
                                    )
                                    first = False
                        yield c, pt

                def scale_wave(key, out_t, psums, gp_t):
                    for c, pt in psums:
                        E(key).tensor_tensor(
                            chunk4(out_t, c),
                            pt[:, :, 0:JC].rearrange(
                                "r m (b j) -> r m b j", j=N_IN
                            ),
                            bcast(gp_t, c), OP.mult,
                        )

                # J1 = W1 @ dh0 ; dh1 = gp1*J1  (overlaps the u-chain below)
                scale_wave("dh1", dh1, mm_wave("j1", w1T, dh0), gp_l[1])

                # ---- u-chain ----
                for m in range(2):
                    up_ = fps.tile([128, BL], F32, tag="a", name="a")
                    for p in range(2):
                        nc.tensor.matmul(
                            up_[:],
                            w2[:, p, m * 128:(m + 1) * 128],
                            gp2u2[:, p, :],
                            start=(p == 0), stop=(p == 1),
                        )
                    nc.vector.tensor_tensor(
                        u1t1[:, m, :], up_[:], t_l[1][:, m, :], OP.mult
                    )
                    nc.vector.tensor_tensor(
                        gp1u1[:, m, :], up_[:], gp_l[1][:, m, :], OP.mult
                    )
                for m in range(2):
                    up_ = fps.tile([128, BL], F32, tag="a", name="a")
                    for p in range(2):
                        nc.tensor.matmul(
                            up_[:],
                            w1[:, p, m * 128:(m + 1) * 128],
                            gp1u1[:, p, :],
                            start=(p == 0), stop=(p == 1),
                        )
                    nc.vector.tensor_tensor(
                        c0[:, m, :], up_[:], tgp0[:, m, :], OP.mult
                    )
                    nc.vector.tensor_tensor(
                        gp0u0[:, m, :], up_[:], gp_l[0][:, m, :], OP.mult
                    )

                # c1*J1 = (u1 t1) * dh1, on GPSIMD (SBUF only)
                for c in range(NCH):
                    E("c1j1").tensor_tensor(
                        chunk4(c1j1, c), chunk4(dh1, c), bcast(u1t1, c), OP.mult
                    )

                # jacobian head via the u-chain: jac = W0^T @ (gp0*u0)
                jp = fps.tile([128, BL], F32, tag="a", name="jp")
                for p in range(2):
                    nc.tensor.matmul(
                        jp[:N_IN, :], w0[:, p, :], gp0u0[:, p, :],
                        start=(p == 0), stop=(p == 1),
                    )
                jacs = outp.tile([N_IN, BL], F32, tag="jacs")
                nc.scalar.activation(jacs[:], jp[:N_IN, :], AF.Copy)
                nc.sync.dma_start(out=d_jac[:], in_=jacs[:])

                # T0[(j,k), b] = W00^T @ c0 (-2 baked into w00 on the host;
                # added into o_hes host-side during unsharding)
                for mt, (m0, msz) in enumerate(((0, 128), (128, NJK - 128))):
                    tp_ = fps.tile([128, BL], F32, tag="a", name="a")
                    for p in range(2):
                        nc.tensor.matmul(
                            tp_[:msz, :], w00[:, p, m0:m0 + msz], c0[:, p, :],
                            start=(p == 0), stop=(p == 1),
                        )
                    t0s = outp.tile([128, BL], F32, tag="t0s", name="t0s")
                    nc.scalar.activation(t0s[:msz, :], tp_[:msz, :], AF.Copy)
                    nc.sync.dma_start(
                        out=d_t0[m0:m0 + msz, :], in_=t0s[:msz, :]
                    )

                # J2 = W2 @ dh1 ; r2 = c2*J2 drained directly
                scale_wave("r2", r2, mm_wave("j2", w2T, dh1), c2h)
                # S2 = W2^T @ r2 ; gs2 = gp1*S2   (reuses dh0's slot)
                gs2 = big.tile([128, 2, BL, N_IN], mm_dt, tag="bigA",
                               name="gs2")
                scale_wave("gs2", gs2, mm_wave("s2", w2, r2), gp_l[1])
                # U1 = c1j1 + gs2 on GPSIMD (keeps S1 single-rhs so the
                # S1 wave runs at DVE-drain pace, not PE pace)
                u1s = big.tile([128, 2, BL, N_IN], mm_dt, tag="c1j1x",
                               name="u1s")
                for c in range(NCH):
                    nc.gpsimd.tensor_tensor(
                        chunk4(u1s, c), chunk4(c1j1, c), chunk4(gs2, c), OP.add
                    )

                # S1 = W1^T @ U1 ; gs1 = gp0*S1 ; H = W0m2^T @ gs1 per chunk
                gs1 = big.tile([128, 2, BL, N_IN], mm_dt, tag="bigB",
                               name="gs1")
                for c, pt in mm_wave("s1", w1, u1s):
                    nc.vector.tensor_tensor(
                        chunk4(gs1, c),
                        pt[:, :, 0:JC].rearrange("r m (b j) -> r m b j", j=N_IN),
                        bcast(gp_l[0], c), OP.mult,
                    )
                    hp = hps.tile([N_IN, 512], F32, tag="h", name="hp")
                    for p in range(2):
                        nc.tensor.matmul(
                            hp[:, 0:JC], w0m2[:, p, :],
                            chunk4(gs1, c)[:, p].rearrange("r b j -> r (b j)"),
                            start=(p == 0), stop=(p == 1),
                        )
                    hs = outp.tile([N_IN, CB, N_IN], F32, tag="hs")
                    nc.scalar.activation(
                        hs[:].rearrange("r b j -> r (b j)"), hp[:, 0:JC],
                        AF.Copy,
                    )
                    nc.sync.dma_start(
                        out=d_hes[:, c * CB:(c + 1) * CB, :], in_=hs[:]
                    )

    nc.compile()
    return nc


_CACHE = {}


def _prep_core_inputs(x, w0, b0, w1, b1, w2, b2, w3, b3):
    """Per-core input maps: core = net * 2 + half."""
    def split(a):  # [256, F] -> [128, 2, F]
        return np.ascontiguousarray(
            a.reshape(2, 128, *a.shape[1:]).transpose(1, 0, *range(2, a.ndim + 1))
        )

    xT = np.ascontiguousarray(x.T)  # [12, 512]
    maps = []
    for n in range(N_NET):
        w00 = -2.0 * w0[n][:, :, None] * w0[n][:, None, :]  # [256, 12, 12]
        per_net = {
            "w0T": np.ascontiguousarray(w0[n].T),
            "w0": split(w0[n]),
            "w1T": split(np.ascontiguousarray(w1[n].T)),
            "w2T": split(np.ascontiguousarray(w2[n].T)),
            "w1": split(w1[n]),
            "w2": split(w2[n]),
            "w3T": split(np.ascontiguousarray(w3[n].T)),
            "w00": split(w00.reshape(W, NJK)),
            "w0m2": split(-2.0 * w0[n]),
            "b0": split(b0[n][:, None]),
            "b1": split(b1[n][:, None]),
            "b2": split(b2[n][:, None]),
            "b3": b3[n][:, None],
        }
        per_net["w0c"] = per_net["w0"]
        per_net["w3c"] = per_net["w3T"]
        for h in range(2):
            m = dict(per_net)
            m["xT"] = np.ascontiguousarray(xT[:, h * BL:(h + 1) * BL])
            maps.append(m)
    return maps


def run(nc, inputs):
    in_maps = _prep_core_inputs(**inputs)
    res = run_bass_kernel_spmd(nc, in_maps, list(range(N_CORES)))
    out = np.zeros((N_NET, B_FULL, 1, 1), np.float32)
    jac = np.zeros((N_NET, B_FULL, 1, N_IN), np.float32)
    hes = np.zeros((N_NET, B_FULL, N_IN, 1, N_IN), np.float32)
    for n in range(N_NET):
        for h in range(2):
            r = res.results[n * 2 + h]
            sl = slice(h * BL, (h + 1) * BL)
            out[n, sl, 0, 0] = r["o_val"][0]
            jac[n, sl, 0, :] = r["o_jac"].T
            hes[n, sl, :, 0, :] = (
                r["o_hes"].transpose(1, 0, 2)
                + r["o_t0"].reshape(N_IN, N_IN, BL).transpose(2, 0, 1)
            )
    return out, jac, hes


def kernel(x, w0, b0, w1, b1, w2, b2, w3, b3):
    inputs = {
        "x": np.asarray(x, np.float32),
        "w0": np.asarray(w0, np.float32), "b0": np.asarray(b0, np.float32),
        "w1": np.asarray(w1, np.float32), "b1": np.asarray(b1, np.float32),
        "w2": np.asarray(w2, np.float32), "b2": np.asarray(b2, np.float32),
        "w3": np.asarray(w3, np.float32), "b3": np.asarray(b3, np.float32),
    }
    if "nc" not in _CACHE:
        _CACHE["nc"] = build()
    return run(_CACHE["nc"], inputs)
